# revision 32
# baseline (speedup 1.0000x reference)
"""CPR linear (int8-dequant matmul with column reordering) on 8 Trainium2
NeuronCores.

Math: y = x[:, col_indices] @ (W_int8 * repeat(scales, gs)) + bias
Equivalently, with inv = argsort(col_indices):
    y[m, n] = sum_j x[m, j] * W[inv[j], n] * scales[inv[j]//gs, n]
so x is consumed in natural column order and the permutation rides on W's
rows (host-side index gather; W is 8x smaller than x).

Sharding: column-parallel. Each core owns 512 output features: its slice
of W (row-permuted, dequantized on host) and bias; x is replicated.

Precision/speed scheme along K (32 k-tiles of 128): NBF tiles run plain
bf16 matmuls; the rest run fp8e4m3 via DoubleRow (2 k-tiles per PE pass).
Optionally NC of the fp8 tiles carry a first-order residual correction
(x8@w8 + dx8@w8 + x8@dw8) packed as extra slots in the same x8/w8
tensors. The host folds a per-column power-of-2 scale 2^(A8+cn) into
every weight format so all products accumulate in one PSUM group;
eviction rescales by 2^-(A8+cn) and adds bias.

HW-measured constraints baked into the schedule (TRN2):
  - a DoubleRow instr after a long bf16 run costs ~170-270c, but only
    ~6 of them per bank; sustained DR streams pay serial LDWEIGHTS
    (~550c) since DoubleRow disables the fast weight load path
  - zero/subnormal fp8 operand values trigger a further ~3x PE slow
    path, so residuals are dithered onto +-2^-6 (smallest e4m3 normal)
  - fp8 operands must live in the same large tiles as the main stream;
    separate small residual tiles measurably run ~3x slower

Per-core device kernel:
  - weights DMA'd into resident tiles, chunked so the first matmuls gate
    on a ~0.25MB load; bias/colscale broadcast [512] -> [128, 512]
  - loop over 8 m-blocks of 1024 rows:
      x loads on the sync queue ([128, t, 1024] tiles per dtype)
      mb 0: k-outer over all 8 PSUM banks so the PE starts on the first
            x granule instead of waiting for the full block
      mb 1+: m-subtile-outer, bf16 run then a trailing DoubleRow burst
      PSUM * colscale + bias -> SBUF pairs -> stores on the gpsimd queue
"""
from contextlib import ExitStack

import numpy as np
import ml_dtypes

import concourse.bass as bass
import concourse.bacc as bacc
import concourse.mybir as mybir
import concourse.tile as tile

B, S, K, N = 4, 2048, 4096, 4096
M = B * S                    # 8192
NCORES = 8
NS = N // NCORES             # 512 output cols per core
P = 128
NKT = K // P                 # 32 k-tiles
MB = 1024                    # m-block rows
NMB = M // MB                # 8
MSUB = MB // P               # 8

# k-tile classes: NBF bf16 tiles + NTRUE fp8 tiles, of which NC carry a
# first-order residual correction. fp8 "slots" = NTRUE + 2*NC rows-of-128
# packed into one x8/w8 tensor pair:
#   [x8 of true tiles | dx8 of corrected | x8-dup of corrected]
#   [w8 of true tiles | w8-dup of corrected | dw8 of corrected]
# so every DoubleRow instr is shape- and tensor-uniform (separate small
# residual tiles measurably hit a PE slow path).
NBF, NTRUE, NC = 24, 8, 0
SLOTS = NTRUE + 2 * NC
assert NBF + NTRUE == NKT and NC <= NTRUE and SLOTS % 2 == 0
A8 = 5                       # x fp8 pre-scale: x*2^5 (max |x|*32 < 240)

bf16 = mybir.dt.bfloat16
f32 = mybir.dt.float32
f8e4 = mybir.dt.float8e4

KB = 4                       # k-tiles per steady-state x DMA


def _granules(n_tiles, first_fine, fine):
    """Chunk n_tiles into DMA granule sizes, finest first."""
    out, left = [], n_tiles
    sched = list(first_fine) if first_fine else []
    for g in sched:
        if left <= 0:
            break
        g = min(g, left)
        out.append(g)
        left -= g
    while left > 0:
        g = min(fine, left)
        out.append(g)
        left -= g
    return out


def build(repeats: int = 1, variant: str = "full"):
    """variant: "full" | "nomm" (DMA path only) | "mmonly" (PE path only)"""
    do_mm = variant in ("full", "mmonly")
    do_xdma = variant in ("full", "nomm")

    nc = bacc.Bacc(None)
    # x pre-transposed on host, split by k-tile class
    x_d = x8_d = dx8_d = None
    w_d = w8_d = dw8_d = None
    if NBF:
        x_d = nc.dram_tensor("xbf", [NBF * P, M], bf16, kind="ExternalInput")
        w_d = nc.dram_tensor("wbf", [NBF * P, NS], bf16, kind="ExternalInput")
    x8_d = nc.dram_tensor("x8", [SLOTS * P, M], f8e4, kind="ExternalInput")
    w8_d = nc.dram_tensor("w8", [SLOTS * P, NS], f8e4, kind="ExternalInput")
    b_d = nc.dram_tensor("bias", [NS], f32, kind="ExternalInput")
    cs_d = nc.dram_tensor("colscale", [NS], f32, kind="ExternalInput")
    y_d = nc.dram_tensor("y", [M, NS], f32, kind="ExternalOutput")

    with tile.TileContext(nc) as tc, ExitStack() as stk:
        if repeats > 1:
            stk.enter_context(tc.For_i(0, repeats, 1))
        with (
            tc.tile_pool(name="consts", bufs=1) as consts,
            tc.tile_pool(name="xpool", bufs=2) as xpool,
            tc.tile_pool(name="opool", bufs=2) as opool,
            tc.tile_pool(name="psum", bufs=1, space="PSUM") as psum_pool,
        ):
            # per-bank op order: a contiguous bf16 run lets the PE weight
            # path run ahead on a trailing DoubleRow burst's LDWEIGHTS, but
            # the run-ahead depth is ~6 instrs — so split into several
            # (bf16 run, <=6-DR burst) rounds
            bf_ops = [("bf", kt) for kt in range(NBF)]
            dr_ops = [("x8", t) for t in range(0, SLOTS, 2)]
            DRB = 6
            rounds = max(1, (len(dr_ops) + DRB - 1) // DRB)
            merged = []
            for r in range(rounds):
                b0 = r * len(bf_ops) // rounds
                b1 = (r + 1) * len(bf_ops) // rounds
                merged += bf_ops[b0:b1] + dr_ops[r * DRB:(r + 1) * DRB]
            n_ops = len(merged)

            # mb0 x-granule schedules (fine so the PE starts early) and
            # their first-use order under `merged`
            gran0 = {"xbf": _granules(NBF, (1, 1, 2), KB) if NBF else [],
                     "x8": _granules(SLOTS, (), 2)}

            def granule_idx(gran, tile_i):
                k0 = 0
                for gi, H in enumerate(gran):
                    if tile_i < k0 + H:
                        return gi
                    k0 += H
                raise AssertionError

            first_use = []      # [(tensor, granule_idx)] in merged order
            seen = set()
            for kind, t in merged:
                needs = ([("xbf", granule_idx(gran0["xbf"], t))]
                         if kind == "bf" else
                         [("x8", granule_idx(gran0["x8"], t)),
                          ("x8", granule_idx(gran0["x8"], t + 1))])
                for need in needs:
                    if need not in seen:
                        seen.add(need)
                        first_use.append(need)
            # any unused granules (shouldn't happen) appended for safety
            for name in ("xbf", "x8"):
                for gi in range(len(gran0[name])):
                    if (name, gi) not in seen:
                        first_use.append((name, gi))

            # resident weights, chunked smallest-first so early matmuls
            # gate on a small load
            def load_w(dst, src, gran):
                k0 = 0
                for H in gran:
                    nc.scalar.dma_start(
                        out=dst[:, k0:k0 + H],
                        in_=src[k0 * P:(k0 + H) * P, :].rearrange(
                            "(t p) n -> p t n", p=P))
                    k0 += H

            wd = (consts.tile([P, NBF, NS], bf16, name="wd")
                  if NBF else None)
            w8t = consts.tile([P, SLOTS, NS], f8e4, name="w8t")

            # weight granules in merged-op first-use order, so the mb0
            # k-outer stream never waits on the weight queue
            wgran = {"xbf": _granules(NBF, (2, 2), 4) if NBF else [],
                     "x8": _granules(SLOTS, (), 2)}
            wtens = {"xbf": (wd, w_d), "x8": (w8t, w8_d)}
            wfirst = []
            wseen = set()
            for kind, t in merged:
                needs = ([("xbf", granule_idx(wgran["xbf"], t))]
                         if kind == "bf" else
                         [("x8", granule_idx(wgran["x8"], t)),
                          ("x8", granule_idx(wgran["x8"], t + 1))])
                for need in needs:
                    if need not in wseen:
                        wseen.add(need)
                        wfirst.append(need)
            for name in ("xbf", "x8"):
                for gi in range(len(wgran[name])):
                    if (name, gi) not in wseen:
                        wfirst.append((name, gi))
            for name, gi in wfirst:
                dst, src = wtens[name]
                k0 = sum(wgran[name][:gi])
                H = wgran[name][gi]
                nc.scalar.dma_start(
                    out=dst[:, k0:k0 + H],
                    in_=src[k0 * P:(k0 + H) * P, :].rearrange(
                        "(t p) n -> p t n", p=P))

            # bias/colscale broadcast to all partitions (needed only at first
            # PSUM eviction, so issued after the W loads on the same queue)
            bias_t = consts.tile([P, NS], f32)
            nc.scalar.dma_start(
                out=bias_t,
                in_=bass.AP(tensor=b_d, offset=0, ap=[[0, P], [1, NS]]),
            )
            cs_t = consts.tile([P, NS], f32)
            nc.scalar.dma_start(
                out=cs_t,
                in_=bass.AP(tensor=cs_d, offset=0, ap=[[0, P], [1, NS]]),
            )

            xbf_s = x8_s = None
            if not do_xdma:
                if NBF:
                    xbf_s = consts.tile([P, NBF, MB], bf16, tag="xbfs")
                    nc.vector.memset(xbf_s, 0.5)
                x8_s = consts.tile([P, SLOTS, MB], f8e4, tag="x8s")
                nc.vector.memset(x8_s, 0.25)

            for mb in range(NMB):
                m0 = mb * MB
                if do_xdma:
                    def alloc_x(name, n_tiles, dt):
                        return xpool.tile([P, n_tiles, MB], dt, tag=name,
                                          name=name)

                    def granule_dma(t, src_d, gran, gi):
                        k0 = sum(gran[:gi])
                        H = gran[gi]
                        src = src_d[k0 * P:(k0 + H) * P, m0:m0 + MB]
                        nc.sync.dma_start(
                            out=t[:, k0:k0 + H],
                            in_=src.rearrange("(b p) m -> p b m", p=P),
                        )

                    xbf = alloc_x("xbf_t", NBF, bf16) if NBF else None
                    x8t = alloc_x("x8_t", SLOTS, f8e4)
                    tens = {"xbf": (xbf, x_d), "x8": (x8t, x8_d)}
                    if mb == 0:
                        # granules in first-use order of the merged op list
                        for name, gi in first_use:
                            t, src_d = tens[name]
                            granule_dma(t, src_d, gran0[name], gi)
                    else:
                        for name in ("xbf", "x8"):
                            t, src_d = tens[name]
                            if t is None:
                                continue
                            n_tiles = {"xbf": NBF, "x8": SLOTS}[name]
                            gran = _granules(n_tiles, (),
                                             KB if name == "xbf" else 8)
                            for gi in range(len(gran)):
                                granule_dma(t, src_d, gran, gi)
                else:
                    xbf, x8t = xbf_s, x8_s
                if not do_mm:
                    continue

                ps = [psum_pool.tile([P, NS], f32, tag=f"ps{ms}",
                                     name=f"ps{ms}")
                      for ms in range(MSUB)]

                def issue_op(op, ms, i):
                    kind, t = op
                    msl = slice(ms * P, (ms + 1) * P)
                    if kind == "bf":
                        x_ap, w_ap, pm = xbf[:, t, msl], wd[:, t], None
                    else:
                        x_ap, w_ap = x8t[:, t:t + 2, msl], w8t[:, t:t + 2, :]
                        pm = mybir.MatmulPerfMode.DoubleRow
                    nc.tensor.matmul(
                        ps[ms], x_ap, w_ap,
                        start=(i == 0), stop=(i == n_ops - 1),
                        perf_mode=pm,
                    )

                if mb == 0:
                    # k-outer across all banks, two merged ops per flush so
                    # bf16/DR still alternate at instruction granularity
                    for i0 in range(0, n_ops, 2):
                        chunk = merged[i0:i0 + 2]
                        for ms in range(MSUB):
                            for j, op in enumerate(chunk):
                                issue_op(op, ms, i0 + j)
                else:
                    for ms in range(MSUB):
                        for i, op in enumerate(merged):
                            issue_op(op, ms, i)

                # evict: y = ps * colscale + bias, pairs -> one 1MB store on
                # the (otherwise idle) gpsimd queue.
                # Last block: per-bank granules to shrink the drain.
                def evict(ms, out_ap):
                    nc.vector.tensor_tensor(
                        out=out_ap, in0=ps[ms], in1=cs_t,
                        op=mybir.AluOpType.mult,
                    )
                    nc.vector.tensor_tensor(
                        out=out_ap, in0=out_ap, in1=bias_t,
                        op=mybir.AluOpType.add,
                    )

                if mb < NMB - 1:
                    for msp in range(MSUB // 2):
                        ot = opool.tile([P, 2, NS], f32, tag="ot")
                        for half in range(2):
                            evict(msp * 2 + half, ot[:, half])
                        row0 = m0 + msp * 2 * P
                        dst = y_d[row0:row0 + 2 * P, :]
                        nc.gpsimd.dma_start(
                            out=dst.rearrange("(b p) n -> p b n", p=P), in_=ot,
                        )
                else:
                    for ms in range(MSUB):
                        ot1 = opool.tile([P, 1, NS], f32, tag="ot1")
                        evict(ms, ot1[:, 0])
                        row0 = m0 + ms * P
                        dst = y_d[row0:row0 + P, :]
                        nc.gpsimd.dma_start(
                            out=dst.rearrange("(b p) n -> p b n", p=P), in_=ot1,
                        )

    nc.compile()
    return nc


def make_in_maps(x, scales, bias, weight_int8, col_indices, group_size):
    """Host-side sharding/layout prep: index gathers, dtype casts, and
    power-of-2 scale folding only."""
    e4 = ml_dtypes.float8_e4m3
    gs = int(group_size)
    x2 = np.asarray(x, dtype=np.float32).reshape(M, K)
    xT = np.ascontiguousarray(x2.T)                      # [K, M]

    ci = np.asarray(col_indices).astype(np.int64)
    inv = np.argsort(ci)                     # inv[j]: W row paired with x col j
    gi = inv // gs                           # scale group per permuted row

    Wp = np.asarray(weight_int8)[inv].astype(np.float32)   # [K, N]
    sc = np.asarray(scales, dtype=np.float32)[gi]          # [K, N] expanded
    wdq = Wp * sc                                          # [K, N] f32
    bias = np.asarray(bias, dtype=np.float32)

    # per-column power-of-2 normalizer: max|wd_n| * 2^cn in (120, 240]
    mxc = np.abs(wdq).max(axis=0)
    cn = np.floor(np.log2(240.0 / np.maximum(mxc, 1e-30))).astype(np.float32)
    cn = np.minimum(cn, 30.0)
    colscale = (2.0 ** -(A8 + cn)).astype(np.float32)

    kb = slice(0, NBF * P)
    k8 = slice(NBF * P, K)

    full = {}
    if NBF:
        full["xbf"] = xT[kb].astype(ml_dtypes.bfloat16)
        full["wbf"] = (wdq[kb] * 2.0 ** (A8 + cn)).astype(ml_dtypes.bfloat16)
    xs = np.clip(xT[k8] * float(2 ** A8), -240, 240)
    x8 = xs.astype(e4)
    ws = wdq[k8] * 2.0 ** cn                       # |ws| <= 240 by cn
    w8 = ws.astype(e4)

    # residual rows for the first NC corrected tiles. Zero/subnormal fp8
    # operands trigger a PE slow path (~3x instruction cost), so dither
    # tiny residuals onto +-2^-6, the smallest e4m3 normal (error impact
    # ~2.6e-4 relative).
    rng = np.random.default_rng(7)

    def dither(v):
        tiny = np.abs(v) < 2.0 ** -6
        signs = rng.integers(0, 2, size=v.shape).astype(np.float32) * 2 - 1
        return np.where(tiny, signs * 2.0 ** -6, v)

    rc = slice(0, NC * P)
    dx8 = dither(xs[rc] - x8[rc].astype(np.float32)).astype(e4)
    dw8 = dither(ws[rc] - w8[rc].astype(np.float32)).astype(e4)
    # slot packing: [true x8 | dx8 | x8-dup] against [true w8 | w8-dup | dw8]
    full["x8"] = np.concatenate([x8, dx8, x8[rc]], axis=0)
    full["w8"] = np.concatenate([w8, w8[rc], dw8], axis=0)

    in_maps = []
    for c in range(NCORES):
        cols = slice(c * NS, (c + 1) * NS)
        m = {k: full[k] for k in ("xbf", "x8") if k in full}
        for k in ("wbf", "w8"):
            if k in full:
                m[k] = np.ascontiguousarray(full[k][:, cols])
        m["bias"] = bias[cols]
        m["colscale"] = colscale[cols]
        in_maps.append(m)
    return in_maps


_RUNNER = None

_REPL = ("xbf", "x8")        # tensors identical on every core


def _make_runner():
    """Build the bass module once and wrap it in a cached sharded jit."""
    import jax
    from jax.sharding import Mesh, PartitionSpec, NamedSharding
    from jax.experimental.shard_map import shard_map
    from concourse import bass2jax
    from concourse.bass2jax import _bass_exec_p, install_neuronx_cc_hook

    nc = build(repeats=1)
    install_neuronx_cc_hook()
    partition_name = nc.partition_id_tensor.name if nc.partition_id_tensor else None

    in_names, out_names, out_avals, zero_outs = [], [], [], []
    for alloc in nc.m.functions[0].allocations:
        if not isinstance(alloc, mybir.MemoryLocationSet):
            continue
        name = alloc.memorylocations[0].name
        if alloc.kind == "ExternalInput":
            if name != partition_name:
                in_names.append(name)
        elif alloc.kind == "ExternalOutput":
            out_names.append(name)
            shape = tuple(alloc.tensor_shape)
            dtype = mybir.dt.np(alloc.dtype)
            out_avals.append(jax.core.ShapedArray(shape, dtype))
            zero_outs.append(np.zeros(shape, dtype))
    all_in_names = list(in_names) + list(out_names)
    if partition_name is not None:
        all_in_names.append(partition_name)
    n_params, n_outs = len(in_names), len(out_names)

    def _body(*args):
        operands = list(args)
        if partition_name is not None:
            operands.append(bass2jax.partition_id_tensor())
        outs = _bass_exec_p.bind(
            *operands,
            out_avals=tuple(out_avals),
            in_names=tuple(all_in_names),
            out_names=tuple(out_names),
            lowering_input_output_aliases=(),
            sim_require_finite=True,
            sim_require_nnan=True,
            nc=nc,
        )
        return tuple(outs)

    devices = jax.devices()[:NCORES]
    mesh = Mesh(np.asarray(devices), ("core",))
    # x tensors are identical on every core: pass them replicated so only one
    # copy crosses the host->device link; per-core tensors are concat-sharded.
    in_specs = tuple(
        PartitionSpec() if name in _REPL else PartitionSpec("core")
        for name in in_names
    ) + (PartitionSpec("core"),) * n_outs
    sharded = jax.jit(
        shard_map(
            _body, mesh=mesh,
            in_specs=in_specs,
            out_specs=(PartitionSpec("core"),) * n_outs,
            check_rep=False,
        ),
        keep_unused=True,
    )
    shard_core = NamedSharding(mesh, PartitionSpec("core"))
    shard_repl = NamedSharding(mesh, PartitionSpec())

    def run(in_maps):
        import jax as _jax
        dev_in = []
        for name in in_names:
            if name in _REPL:
                dev_in.append(
                    _jax.device_put(np.asarray(in_maps[0][name]), shard_repl))
            else:
                a = np.concatenate(
                    [np.asarray(in_maps[c][name]) for c in range(NCORES)], axis=0)
                dev_in.append(_jax.device_put(a, shard_core))
        dev_zero = [
            _jax.device_put(
                np.zeros((NCORES * z.shape[0], *z.shape[1:]), z.dtype), shard_core)
            for z in zero_outs
        ]
        out = sharded(*dev_in, *dev_zero)
        return [
            {name: np.asarray(out[i]).reshape(NCORES, *zero_outs[i].shape)[c]
             for i, name in enumerate(out_names)}
            for c in range(NCORES)
        ]

    return run


def kernel(x, scales, bias, weight_int8, col_indices, group_size):
    global _RUNNER
    in_maps = make_in_maps(x, scales, bias, weight_int8, col_indices, group_size)
    if _RUNNER is None:
        _RUNNER = _make_runner()
    results = _RUNNER(in_maps)
    y = np.concatenate([results[c]["y"] for c in range(NCORES)], axis=1)
    return np.ascontiguousarray(y.reshape(B, S, N))


# revision 33
# speedup vs baseline: 1.0337x; 1.0337x over previous
"""CPR linear (int8-dequant matmul with column reordering) on 8 Trainium2
NeuronCores.

Math: y = x[:, col_indices] @ (W_int8 * repeat(scales, gs)) + bias
Equivalently, with inv = argsort(col_indices):
    y[m, n] = sum_j x[m, j] * W[inv[j], n] * scales[inv[j]//gs, n]
so x is consumed in natural column order and the permutation rides on W's
rows (host-side index gather; W is 8x smaller than x).

Sharding: column-parallel. Each core owns 512 output features: its slice
of W (row-permuted, dequantized on host) and bias; x is replicated.

Precision/speed scheme along K (32 k-tiles of 128): NBF tiles run plain
bf16 matmuls; the rest run fp8e4m3 via DoubleRow (2 k-tiles per PE pass).
Optionally NC of the fp8 tiles carry a first-order residual correction
(x8@w8 + dx8@w8 + x8@dw8) packed as extra slots in the same x8/w8
tensors. The host folds a per-column power-of-2 scale 2^(A8+cn) into
every weight format so all products accumulate in one PSUM group;
eviction rescales by 2^-(A8+cn) and adds bias.

HW-measured constraints baked into the schedule (TRN2):
  - a DoubleRow instr after a long bf16 run costs ~170-270c, but only
    ~6 of them per bank; sustained DR streams pay serial LDWEIGHTS
    (~550c) since DoubleRow disables the fast weight load path
  - zero/subnormal fp8 operand values trigger a further ~3x PE slow
    path, so residuals are dithered onto +-2^-6 (smallest e4m3 normal)
  - fp8 operands must live in the same large tiles as the main stream;
    separate small residual tiles measurably run ~3x slower

Per-core device kernel:
  - weights DMA'd into resident tiles, chunked so the first matmuls gate
    on a ~0.25MB load; bias/colscale broadcast [512] -> [128, 512]
  - loop over 8 m-blocks of 1024 rows:
      x loads on the sync queue ([128, t, 1024] tiles per dtype)
      mb 0: k-outer over all 8 PSUM banks so the PE starts on the first
            x granule instead of waiting for the full block
      mb 1+: m-subtile-outer, bf16 run then a trailing DoubleRow burst
      PSUM * colscale + bias -> SBUF pairs -> stores on the gpsimd queue
"""
from contextlib import ExitStack

import numpy as np
import ml_dtypes

import concourse.bass as bass
import concourse.bacc as bacc
import concourse.mybir as mybir
import concourse.tile as tile

B, S, K, N = 4, 2048, 4096, 4096
M = B * S                    # 8192
NCORES = 8
NS = N // NCORES             # 512 output cols per core
P = 128
NKT = K // P                 # 32 k-tiles
MB = 1024                    # m-block rows
NMB = M // MB                # 8
MSUB = MB // P               # 8

# k-tile classes: NBF bf16 tiles + NTRUE fp8 tiles, of which NC carry a
# first-order residual correction. fp8 "slots" = NTRUE + 2*NC rows-of-128
# packed into one x8/w8 tensor pair:
#   [x8 of true tiles | dx8 of corrected | x8-dup of corrected]
#   [w8 of true tiles | w8-dup of corrected | dw8 of corrected]
# so every DoubleRow instr is shape- and tensor-uniform (separate small
# residual tiles measurably hit a PE slow path).
NBF, NTRUE, NC = 26, 6, 0
SLOTS = NTRUE + 2 * NC
assert NBF + NTRUE == NKT and NC <= NTRUE and SLOTS % 2 == 0
A8 = 5                       # x fp8 pre-scale: x*2^5 (max |x|*32 < 240)

bf16 = mybir.dt.bfloat16
f32 = mybir.dt.float32
f8e4 = mybir.dt.float8e4

KB = 4                       # k-tiles per steady-state x DMA


def _granules(n_tiles, first_fine, fine):
    """Chunk n_tiles into DMA granule sizes, finest first."""
    out, left = [], n_tiles
    sched = list(first_fine) if first_fine else []
    for g in sched:
        if left <= 0:
            break
        g = min(g, left)
        out.append(g)
        left -= g
    while left > 0:
        g = min(fine, left)
        out.append(g)
        left -= g
    return out


def build(repeats: int = 1, variant: str = "full"):
    """variant: "full" | "nomm" (DMA path only) | "mmonly" (PE path only)"""
    do_mm = variant in ("full", "mmonly")
    do_xdma = variant in ("full", "nomm")

    nc = bacc.Bacc(None)
    # x pre-transposed on host, split by k-tile class
    x_d = x8_d = dx8_d = None
    w_d = w8_d = dw8_d = None
    if NBF:
        x_d = nc.dram_tensor("xbf", [NBF * P, M], bf16, kind="ExternalInput")
        w_d = nc.dram_tensor("wbf", [NBF * P, NS], bf16, kind="ExternalInput")
    x8_d = nc.dram_tensor("x8", [SLOTS * P, M], f8e4, kind="ExternalInput")
    w8_d = nc.dram_tensor("w8", [SLOTS * P, NS], f8e4, kind="ExternalInput")
    b_d = nc.dram_tensor("bias", [NS], f32, kind="ExternalInput")
    cs_d = nc.dram_tensor("colscale", [NS], f32, kind="ExternalInput")
    y_d = nc.dram_tensor("y", [M, NS], f32, kind="ExternalOutput")

    with tile.TileContext(nc) as tc, ExitStack() as stk:
        if repeats > 1:
            stk.enter_context(tc.For_i(0, repeats, 1))
        with (
            tc.tile_pool(name="consts", bufs=1) as consts,
            tc.tile_pool(name="xpool", bufs=2) as xpool,
            tc.tile_pool(name="opool", bufs=2) as opool,
            tc.tile_pool(name="psum", bufs=1, space="PSUM") as psum_pool,
        ):
            # per-bank op order: a contiguous bf16 run lets the PE weight
            # path run ahead on a trailing DoubleRow burst's LDWEIGHTS, but
            # the run-ahead depth is ~6 instrs — so split into several
            # (bf16 run, <=6-DR burst) rounds
            bf_ops = [("bf", kt) for kt in range(NBF)]
            dr_ops = [("x8", t) for t in range(0, SLOTS, 2)]
            DRB = 6
            rounds = max(1, (len(dr_ops) + DRB - 1) // DRB)
            merged = []
            for r in range(rounds):
                b0 = r * len(bf_ops) // rounds
                b1 = (r + 1) * len(bf_ops) // rounds
                merged += bf_ops[b0:b1] + dr_ops[r * DRB:(r + 1) * DRB]
            n_ops = len(merged)

            # mb0 x-granule schedules (fine so the PE starts early) and
            # their first-use order under `merged`
            gran0 = {"xbf": _granules(NBF, (1, 1, 2), KB) if NBF else [],
                     "x8": _granules(SLOTS, (), 2)}

            def granule_idx(gran, tile_i):
                k0 = 0
                for gi, H in enumerate(gran):
                    if tile_i < k0 + H:
                        return gi
                    k0 += H
                raise AssertionError

            first_use = []      # [(tensor, granule_idx)] in merged order
            seen = set()
            for kind, t in merged:
                needs = ([("xbf", granule_idx(gran0["xbf"], t))]
                         if kind == "bf" else
                         [("x8", granule_idx(gran0["x8"], t)),
                          ("x8", granule_idx(gran0["x8"], t + 1))])
                for need in needs:
                    if need not in seen:
                        seen.add(need)
                        first_use.append(need)
            # any unused granules (shouldn't happen) appended for safety
            for name in ("xbf", "x8"):
                for gi in range(len(gran0[name])):
                    if (name, gi) not in seen:
                        first_use.append((name, gi))

            # resident weights, chunked smallest-first so early matmuls
            # gate on a small load
            def load_w(dst, src, gran):
                k0 = 0
                for H in gran:
                    nc.scalar.dma_start(
                        out=dst[:, k0:k0 + H],
                        in_=src[k0 * P:(k0 + H) * P, :].rearrange(
                            "(t p) n -> p t n", p=P))
                    k0 += H

            wd = (consts.tile([P, NBF, NS], bf16, name="wd")
                  if NBF else None)
            w8t = consts.tile([P, SLOTS, NS], f8e4, name="w8t")

            # weight granules in merged-op first-use order, so the mb0
            # k-outer stream never waits on the weight queue
            wgran = {"xbf": _granules(NBF, (2, 2), 4) if NBF else [],
                     "x8": _granules(SLOTS, (), 2)}
            wtens = {"xbf": (wd, w_d), "x8": (w8t, w8_d)}
            wfirst = []
            wseen = set()
            for kind, t in merged:
                needs = ([("xbf", granule_idx(wgran["xbf"], t))]
                         if kind == "bf" else
                         [("x8", granule_idx(wgran["x8"], t)),
                          ("x8", granule_idx(wgran["x8"], t + 1))])
                for need in needs:
                    if need not in wseen:
                        wseen.add(need)
                        wfirst.append(need)
            for name in ("xbf", "x8"):
                for gi in range(len(wgran[name])):
                    if (name, gi) not in wseen:
                        wfirst.append((name, gi))
            for name, gi in wfirst:
                dst, src = wtens[name]
                k0 = sum(wgran[name][:gi])
                H = wgran[name][gi]
                nc.scalar.dma_start(
                    out=dst[:, k0:k0 + H],
                    in_=src[k0 * P:(k0 + H) * P, :].rearrange(
                        "(t p) n -> p t n", p=P))

            # bias/colscale broadcast to all partitions (needed only at first
            # PSUM eviction, so issued after the W loads on the same queue)
            bias_t = consts.tile([P, NS], f32)
            nc.scalar.dma_start(
                out=bias_t,
                in_=bass.AP(tensor=b_d, offset=0, ap=[[0, P], [1, NS]]),
            )
            cs_t = consts.tile([P, NS], f32)
            nc.scalar.dma_start(
                out=cs_t,
                in_=bass.AP(tensor=cs_d, offset=0, ap=[[0, P], [1, NS]]),
            )

            xbf_s = x8_s = None
            if not do_xdma:
                if NBF:
                    xbf_s = consts.tile([P, NBF, MB], bf16, tag="xbfs")
                    nc.vector.memset(xbf_s, 0.5)
                x8_s = consts.tile([P, SLOTS, MB], f8e4, tag="x8s")
                nc.vector.memset(x8_s, 0.25)

            for mb in range(NMB):
                m0 = mb * MB
                if do_xdma:
                    def alloc_x(name, n_tiles, dt):
                        return xpool.tile([P, n_tiles, MB], dt, tag=name,
                                          name=name)

                    def granule_dma(t, src_d, gran, gi):
                        k0 = sum(gran[:gi])
                        H = gran[gi]
                        src = src_d[k0 * P:(k0 + H) * P, m0:m0 + MB]
                        nc.sync.dma_start(
                            out=t[:, k0:k0 + H],
                            in_=src.rearrange("(b p) m -> p b m", p=P),
                        )

                    xbf = alloc_x("xbf_t", NBF, bf16) if NBF else None
                    x8t = alloc_x("x8_t", SLOTS, f8e4)
                    tens = {"xbf": (xbf, x_d), "x8": (x8t, x8_d)}
                    if mb == 0:
                        # granules in first-use order of the merged op list
                        for name, gi in first_use:
                            t, src_d = tens[name]
                            granule_dma(t, src_d, gran0[name], gi)
                    else:
                        for name in ("xbf", "x8"):
                            t, src_d = tens[name]
                            if t is None:
                                continue
                            n_tiles = {"xbf": NBF, "x8": SLOTS}[name]
                            gran = _granules(n_tiles, (),
                                             KB if name == "xbf" else 8)
                            for gi in range(len(gran)):
                                granule_dma(t, src_d, gran, gi)
                else:
                    xbf, x8t = xbf_s, x8_s
                if not do_mm:
                    continue

                ps = [psum_pool.tile([P, NS], f32, tag=f"ps{ms}",
                                     name=f"ps{ms}")
                      for ms in range(MSUB)]

                def issue_op(op, ms, i):
                    kind, t = op
                    msl = slice(ms * P, (ms + 1) * P)
                    if kind == "bf":
                        x_ap, w_ap, pm = xbf[:, t, msl], wd[:, t], None
                    else:
                        x_ap, w_ap = x8t[:, t:t + 2, msl], w8t[:, t:t + 2, :]
                        pm = mybir.MatmulPerfMode.DoubleRow
                    nc.tensor.matmul(
                        ps[ms], x_ap, w_ap,
                        start=(i == 0), stop=(i == n_ops - 1),
                        perf_mode=pm,
                    )

                if mb == 0:
                    # k-outer across all banks, two merged ops per flush so
                    # bf16/DR still alternate at instruction granularity
                    for i0 in range(0, n_ops, 2):
                        chunk = merged[i0:i0 + 2]
                        for ms in range(MSUB):
                            for j, op in enumerate(chunk):
                                issue_op(op, ms, i0 + j)
                else:
                    for ms in range(MSUB):
                        for i, op in enumerate(merged):
                            issue_op(op, ms, i)

                # evict: y = ps * colscale + bias, pairs -> one 1MB store on
                # the (otherwise idle) gpsimd queue.
                # Last block: per-bank granules to shrink the drain.
                def evict(ms, out_ap):
                    nc.vector.tensor_tensor(
                        out=out_ap, in0=ps[ms], in1=cs_t,
                        op=mybir.AluOpType.mult,
                    )
                    nc.vector.tensor_tensor(
                        out=out_ap, in0=out_ap, in1=bias_t,
                        op=mybir.AluOpType.add,
                    )

                if mb < NMB - 1:
                    for msp in range(MSUB // 2):
                        ot = opool.tile([P, 2, NS], f32, tag="ot")
                        for half in range(2):
                            evict(msp * 2 + half, ot[:, half])
                        row0 = m0 + msp * 2 * P
                        dst = y_d[row0:row0 + 2 * P, :]
                        nc.gpsimd.dma_start(
                            out=dst.rearrange("(b p) n -> p b n", p=P), in_=ot,
                        )
                else:
                    for ms in range(MSUB):
                        ot1 = opool.tile([P, 1, NS], f32, tag="ot1")
                        evict(ms, ot1[:, 0])
                        row0 = m0 + ms * P
                        dst = y_d[row0:row0 + P, :]
                        nc.gpsimd.dma_start(
                            out=dst.rearrange("(b p) n -> p b n", p=P), in_=ot1,
                        )

    nc.compile()
    return nc


def make_in_maps(x, scales, bias, weight_int8, col_indices, group_size):
    """Host-side sharding/layout prep: index gathers, dtype casts, and
    power-of-2 scale folding only."""
    e4 = ml_dtypes.float8_e4m3
    gs = int(group_size)
    x2 = np.asarray(x, dtype=np.float32).reshape(M, K)
    xT = np.ascontiguousarray(x2.T)                      # [K, M]

    ci = np.asarray(col_indices).astype(np.int64)
    inv = np.argsort(ci)                     # inv[j]: W row paired with x col j
    gi = inv // gs                           # scale group per permuted row

    Wp = np.asarray(weight_int8)[inv].astype(np.float32)   # [K, N]
    sc = np.asarray(scales, dtype=np.float32)[gi]          # [K, N] expanded
    wdq = Wp * sc                                          # [K, N] f32
    bias = np.asarray(bias, dtype=np.float32)

    # per-column power-of-2 normalizer: max|wd_n| * 2^cn in (120, 240]
    mxc = np.abs(wdq).max(axis=0)
    cn = np.floor(np.log2(240.0 / np.maximum(mxc, 1e-30))).astype(np.float32)
    cn = np.minimum(cn, 30.0)
    colscale = (2.0 ** -(A8 + cn)).astype(np.float32)

    kb = slice(0, NBF * P)
    k8 = slice(NBF * P, K)

    full = {}
    if NBF:
        full["xbf"] = xT[kb].astype(ml_dtypes.bfloat16)
        full["wbf"] = (wdq[kb] * 2.0 ** (A8 + cn)).astype(ml_dtypes.bfloat16)
    xs = np.clip(xT[k8] * float(2 ** A8), -240, 240)
    x8 = xs.astype(e4)
    ws = wdq[k8] * 2.0 ** cn                       # |ws| <= 240 by cn
    w8 = ws.astype(e4)

    # residual rows for the first NC corrected tiles. Zero/subnormal fp8
    # operands trigger a PE slow path (~3x instruction cost), so dither
    # tiny residuals onto +-2^-6, the smallest e4m3 normal (error impact
    # ~2.6e-4 relative).
    rng = np.random.default_rng(7)

    def dither(v):
        tiny = np.abs(v) < 2.0 ** -6
        signs = rng.integers(0, 2, size=v.shape).astype(np.float32) * 2 - 1
        return np.where(tiny, signs * 2.0 ** -6, v)

    rc = slice(0, NC * P)
    dx8 = dither(xs[rc] - x8[rc].astype(np.float32)).astype(e4)
    dw8 = dither(ws[rc] - w8[rc].astype(np.float32)).astype(e4)
    # slot packing: [true x8 | dx8 | x8-dup] against [true w8 | w8-dup | dw8]
    full["x8"] = np.concatenate([x8, dx8, x8[rc]], axis=0)
    full["w8"] = np.concatenate([w8, w8[rc], dw8], axis=0)

    in_maps = []
    for c in range(NCORES):
        cols = slice(c * NS, (c + 1) * NS)
        m = {k: full[k] for k in ("xbf", "x8") if k in full}
        for k in ("wbf", "w8"):
            if k in full:
                m[k] = np.ascontiguousarray(full[k][:, cols])
        m["bias"] = bias[cols]
        m["colscale"] = colscale[cols]
        in_maps.append(m)
    return in_maps


_RUNNER = None

_REPL = ("xbf", "x8")        # tensors identical on every core


def _make_runner():
    """Build the bass module once and wrap it in a cached sharded jit."""
    import jax
    from jax.sharding import Mesh, PartitionSpec, NamedSharding
    from jax.experimental.shard_map import shard_map
    from concourse import bass2jax
    from concourse.bass2jax import _bass_exec_p, install_neuronx_cc_hook

    nc = build(repeats=1)
    install_neuronx_cc_hook()
    partition_name = nc.partition_id_tensor.name if nc.partition_id_tensor else None

    in_names, out_names, out_avals, zero_outs = [], [], [], []
    for alloc in nc.m.functions[0].allocations:
        if not isinstance(alloc, mybir.MemoryLocationSet):
            continue
        name = alloc.memorylocations[0].name
        if alloc.kind == "ExternalInput":
            if name != partition_name:
                in_names.append(name)
        elif alloc.kind == "ExternalOutput":
            out_names.append(name)
            shape = tuple(alloc.tensor_shape)
            dtype = mybir.dt.np(alloc.dtype)
            out_avals.append(jax.core.ShapedArray(shape, dtype))
            zero_outs.append(np.zeros(shape, dtype))
    all_in_names = list(in_names) + list(out_names)
    if partition_name is not None:
        all_in_names.append(partition_name)
    n_params, n_outs = len(in_names), len(out_names)

    def _body(*args):
        operands = list(args)
        if partition_name is not None:
            operands.append(bass2jax.partition_id_tensor())
        outs = _bass_exec_p.bind(
            *operands,
            out_avals=tuple(out_avals),
            in_names=tuple(all_in_names),
            out_names=tuple(out_names),
            lowering_input_output_aliases=(),
            sim_require_finite=True,
            sim_require_nnan=True,
            nc=nc,
        )
        return tuple(outs)

    devices = jax.devices()[:NCORES]
    mesh = Mesh(np.asarray(devices), ("core",))
    # x tensors are identical on every core: pass them replicated so only one
    # copy crosses the host->device link; per-core tensors are concat-sharded.
    in_specs = tuple(
        PartitionSpec() if name in _REPL else PartitionSpec("core")
        for name in in_names
    ) + (PartitionSpec("core"),) * n_outs
    sharded = jax.jit(
        shard_map(
            _body, mesh=mesh,
            in_specs=in_specs,
            out_specs=(PartitionSpec("core"),) * n_outs,
            check_rep=False,
        ),
        keep_unused=True,
    )
    shard_core = NamedSharding(mesh, PartitionSpec("core"))
    shard_repl = NamedSharding(mesh, PartitionSpec())

    def run(in_maps):
        import jax as _jax
        dev_in = []
        for name in in_names:
            if name in _REPL:
                dev_in.append(
                    _jax.device_put(np.asarray(in_maps[0][name]), shard_repl))
            else:
                a = np.concatenate(
                    [np.asarray(in_maps[c][name]) for c in range(NCORES)], axis=0)
                dev_in.append(_jax.device_put(a, shard_core))
        dev_zero = [
            _jax.device_put(
                np.zeros((NCORES * z.shape[0], *z.shape[1:]), z.dtype), shard_core)
            for z in zero_outs
        ]
        out = sharded(*dev_in, *dev_zero)
        return [
            {name: np.asarray(out[i]).reshape(NCORES, *zero_outs[i].shape)[c]
             for i, name in enumerate(out_names)}
            for c in range(NCORES)
        ]

    return run


def kernel(x, scales, bias, weight_int8, col_indices, group_size):
    global _RUNNER
    in_maps = make_in_maps(x, scales, bias, weight_int8, col_indices, group_size)
    if _RUNNER is None:
        _RUNNER = _make_runner()
    results = _RUNNER(in_maps)
    y = np.concatenate([results[c]["y"] for c in range(NCORES)], axis=1)
    return np.ascontiguousarray(y.reshape(B, S, N))
